# revision 13
# baseline (speedup 1.0000x reference)
"""Trainium2 Bass kernel for nn_DingoNet_76854144795142 (v2).

Pipeline (per core, 16 of 128 batches, pure data parallel):
  seq -> one-hot fp8e4 (exact 0/1) from host -> conv1d as 3 accumulating
  bf16xfp8 PE matmuls from per-tap tables G_k = conv_w[:,:,k] @ emb.T
  (16-way tile_position packing: 4 batches x 4 sections on the 128x128 array)
  -> relu evac (DVE + GpSimd split) -> H bf16 (d-major, section-blocked
  permutation d' = s*32+c)
  -> T = W_a' @ H bf16 (stationary weights, PSUM [128,1024]) -> tanh (ACT,
  1024-wide instrs) -> tt bf16
  -> e = v' @ tt (M=32 matmuls, 4-way column packing) -> exp folded into
  PSUM evac with accum_out => softmax partial sums
  -> expe broadcast to 128 partitions via SBUF->SBUF stride-0 DMA
  -> ctx via single fused DVE tensor_tensor_reduce (bf16 mult +
  free-dim accumulate over all 2048 positions)
  -> normalization + inverse permutation on host.

Weight-only tables (G_k, permuted W_a / v) are precomputed on host.
"""

import os
import sys

import numpy as np

for _p in ("/opt/trn_rl_repo", "/root/.axon_site/_ro/trn_rl_repo"):
    if os.path.isdir(_p) and _p not in sys.path:
        sys.path.insert(0, _p)

# ---- problem constants (hardcoded per task contract) ----
B = 128          # total batch
L = 8194         # sequence length
PL = 8192        # conv output length
S = 4            # sections
NL = 2048        # positions per section
C = 32           # conv channels
V = 26           # vocab
D = 128          # C * S
NCORES = 8
BPC = B // NCORES      # 16 batches per core
NG = BPC // 4          # 4 groups of 4 batches
CHUNK = 512            # free-dim chunk
NCHUNK = NL // CHUNK   # 4 chunks per section
GWIN = NL + 2          # per-group section window incl. conv taps

_CACHE = {}


def _build_program():
    """Build + compile the Bass/Tile program once per process."""
    if "prog" in _CACHE:
        return _CACHE["prog"]

    from contextlib import ExitStack

    import concourse.bass as bass
    import concourse.tile as tile
    from concourse import bacc, mybir

    f32 = mybir.dt.float32
    bf16 = mybir.dt.bfloat16
    fp8 = mybir.dt.float8e4
    AF = mybir.ActivationFunctionType
    ALU = mybir.AluOpType

    nc = bacc.Bacc(
        "TRN2",
        target_bir_lowering=False,
        debug=False,
        enable_asserts=True,
        num_devices=NCORES,
    )

    oh_d = nc.dram_tensor("oh", [BPC, 32, L], fp8, kind="ExternalInput")
    gt_d = nc.dram_tensor("gt", [128, 3, 32], bf16, kind="ExternalInput")
    wat_d = nc.dram_tensor("wat", [128, 128], bf16, kind="ExternalInput")
    vcol_d = nc.dram_tensor("vcol", [128, 32], bf16, kind="ExternalInput")
    ctxu_d = nc.dram_tensor("ctxu", [128, BPC], f32, kind="ExternalOutput")
    zsum_d = nc.dram_tensor("zsum", [BPC, 1], f32, kind="ExternalOutput")

    with tile.TileContext(nc) as tc, ExitStack() as ctx:
        consts = ctx.enter_context(tc.tile_pool(name="consts", bufs=1))
        ohp = ctx.enter_context(tc.tile_pool(name="ohp", bufs=2))
        hpool = ctx.enter_context(tc.tile_pool(name="hpool", bufs=3))
        ttpool = ctx.enter_context(tc.tile_pool(name="ttpool", bufs=3))
        eepool = ctx.enter_context(tc.tile_pool(name="eepool", bufs=2))
        ebcp = ctx.enter_context(tc.tile_pool(name="ebcp", bufs=3))
        scrp = ctx.enter_context(tc.tile_pool(name="scrp", bufs=2))
        smallp = ctx.enter_context(tc.tile_pool(name="smallp", bufs=2))
        psum_y = ctx.enter_context(tc.tile_pool(name="psum_y", bufs=1, space="PSUM"))
        psum_t = ctx.enter_context(tc.tile_pool(name="psum_t", bufs=1, space="PSUM"))
        psum_e = ctx.enter_context(tc.tile_pool(name="psum_e", bufs=1, space="PSUM"))

        gt_sb = consts.tile([128, 3, 32], bf16)
        nc.sync.dma_start(gt_sb[:, :, :], gt_d.ap())
        wat_sb = consts.tile([128, 128], bf16)
        nc.sync.dma_start(wat_sb[:, :], wat_d.ap())
        vcol_sb = consts.tile([128, 32], bf16)
        nc.sync.dma_start(vcol_sb[:, :], vcol_d.ap())

        oh_ap = oh_d.ap()

        def strided4(t, fr=None):
            # view of tile t at partitions {0,32,64,96}: SBUF AP dim0 step is
            # the per-partition address pitch, so stride-32 partitions =
            # 32x the row pitch.
            a = t[:, :] if fr is None else t[:, fr]
            pitch = a.ap[0][0]
            return bass.AP(
                tensor=a.tensor,
                offset=a.offset,
                ap=[[pitch * 32, 4]] + list(a.ap[1:]),
            )

        def emit_oh_load(g):
            # one-hot for group g: [128 = 4 batches x 32 syms, S, GWIN] fp8
            oh = ohp.tile([128, S, GWIN], fp8, tag="oh", name="oh")
            for j in range(S):
                src = bass.AP(
                    tensor=oh_ap.tensor,
                    offset=(4 * g) * 32 * L + NL * j,
                    ap=[[32 * L, 4], [L, 32], [1, GWIN]],
                )
                nc.sync.dma_start(out=oh[:, j, :], in_=src)
            return oh

        def emit_conv_chunk(g, q, oh, hpq):
            # 48 packed bf16xfp8 matmuls -> 4 PSUM tiles (one per batch)
            yps = [
                psum_y.tile([128, 2 * CHUNK], f32, tag=f"yp{_i}", name=f"yp{_i}")
                for _i in range(2)
            ]
            for k in range(3):
                for j in range(S):
                    for i in range(4):
                        nc.tensor.matmul(
                            out=yps[i // 2][32 * j : 32 * j + 32, CHUNK * (i % 2) : CHUNK * (i % 2 + 1)],
                            lhsT=gt_sb[32 * i : 32 * i + 32, k, :],
                            rhs=oh[32 * i : 32 * i + 32, j, CHUNK * q + k : CHUNK * q + k + CHUNK],
                            start=(k == 0),
                            stop=(k == 2),
                            skip_group_check=True,
                            tile_position=(32 * i, 32 * j),
                        )
            # relu evac PSUM -> H bf16, paired [128,1024] instrs
            # (H tiles for the pair sit adjacently in one hpair tile)
            for p2 in range(2):
                dst = hpq[p2][:, q, :, :]
                nc.vector.tensor_scalar_max(dst, yps[p2][:, :], 0.0)

        tps = {}

        def emit_T_chunk(g, q, hpq):
            # 4 T-matmuls (one per batch) -> [128, 1024] PSUM halves; tanh
            # evac at 1024 granularity -> tt bf16
            tts = []
            for h in range(2):
                tp = psum_t.tile([128, 2 * CHUNK], f32, tag="tp", name="tp")
                for hh in range(2):
                    i = 2 * h + hh
                    nc.tensor.matmul(
                        out=tp[:, CHUNK * hh : CHUNK * (hh + 1)],
                        lhsT=wat_sb[:, :],
                        rhs=hpq[i // 2][:, q, i % 2, :],
                        start=True,
                        stop=True,
                    )
                tt = ttpool.tile([128, 2 * CHUNK], bf16, tag=f"tt{h}", name=f"tt{h}")
                nc.scalar.activation(tt[:, :], tp[:, :], AF.Tanh)
                tts.append(tt)
            return tts

        def emit_e_chunk(g, q, tts, ep, ee, zpart):
            # 4-way column-packed e matmuls into ep half (chunk parity q%2)
            half = q % 2
            for j in range(4):
                nc.tensor.matmul(
                    out=ep[32 * j : 32 * j + 32, CHUNK * half : CHUNK * (half + 1)],
                    lhsT=vcol_sb[:, :],
                    rhs=tts[j // 2][:, CHUNK * (j % 2) : CHUNK * (j % 2 + 1)],
                    start=True,
                    stop=True,
                    tile_position=(0, 32 * j),
                )
            if half == 1:
                # exp over both chunks at once + softmax partial sums
                qq = q // 2
                nc.scalar.activation(
                    ee[:, 2 * CHUNK * qq : 2 * CHUNK * (qq + 1)],
                    ep[:, :],
                    AF.Exp,
                    accum_out=zpart[:, qq : qq + 1],
                )

        def emit_group_tail(g, ee, zpart):
            zsum_sb = smallp.tile([128, 1], f32, tag="zsum", name="zsum")
            zscr = smallp.tile([128, 2], f32, tag="zscr", name="zscr")
            nc.scalar.activation(
                zscr[:, :], zpart[:, :], AF.Copy, accum_out=zsum_sb[:, :]
            )
            nc.scalar.dma_start(
                out=zsum_d.ap()[4 * g : 4 * g + 4, :], in_=strided4(zsum_sb)
            )

        ebs = {}

        def emit_eb_prefetch(v, ee):
            # broadcast expe of batch v to all 128 partitions: the packed
            # e-matmul already replicated it across its own 32-partition
            # block, so this is 4 plain SBUF->SBUF block copies.
            g, i = divmod(v, 4)
            eb = ebcp.tile([128, NL], bf16, tag="eb", name="eb")
            for r in range(4):
                nc.sync.dma_start(
                    out=eb[32 * r : 32 * r + 32, :], in_=ee[32 * i : 32 * i + 32, :]
                )
            ebs[v] = eb

        def emit_ctx_batch(g, i, hpq, ctxg):
            eb = ebs.pop(4 * g + i)
            h_i = hpq[i // 2][:, :, i % 2, :]  # [128, NCHUNK, CHUNK] strided
            scr = scrp.tile([128, NL], bf16, tag="scr", name="scr")
            eb3 = eb[:, :].rearrange("p (a b) -> p a b", a=NCHUNK)
            scr3 = scr[:, :].rearrange("p (a b) -> p a b", a=NCHUNK)
            if g != 2:
                # experiment A: fused scalar_tensor_tensor (1 pass)
                nc.vector.scalar_tensor_tensor(
                    out=scr3,
                    in0=h_i,
                    scalar=1.0,
                    in1=eb3,
                    op0=ALU.mult,
                    op1=ALU.mult,
                    accum_out=ctxg[:, i : i + 1],
                )
            else:
                # experiment B: 2-pass tensor_tensor + tensor_scalar accum
                nc.vector.tensor_tensor(scr3, h_i, eb3, ALU.mult)
                scr2 = scrp.tile([128, NL], bf16, tag="scr2", name="scr2")
                nc.vector.tensor_scalar(
                    scr2[:, :], scr[:, :], 1.0, None, ALU.mult, ALU.add,
                    accum_out=ctxg[:, i : i + 1],
                )


        NU = NG * NCHUNK  # 16 conv units
        ohs = {}
        hps = {}
        tts = {}
        eps = {}
        ees = {}
        zparts = {}
        ctxgs = {}

        def unit_conv(u):
            g, q = divmod(u, NCHUNK)
            if q == 0:
                ohs[g] = emit_oh_load(g)
                hps[g] = [
                    hpool.tile([128, NCHUNK, 2, CHUNK], bf16, tag=f"hp{_i}", name=f"hp{_i}")
                    for _i in range(2)
                ]
                ees[g] = eepool.tile([128, NL], bf16, tag="ee", name="ee")
                zparts[g] = smallp.tile([128, 2], f32, tag="zpart", name="zpart")
            emit_conv_chunk(g, q, ohs[g], hps[g])

        def unit_T(u):
            g, q = divmod(u, NCHUNK)
            tts[(g, q)] = emit_T_chunk(g, q, hps[g])

        def unit_e(u):
            g, q = divmod(u, NCHUNK)
            if q % 2 == 0:
                eps[g] = psum_e.tile([128, 2 * CHUNK], f32, tag="ep", name="ep")
            emit_e_chunk(g, q, tts.pop((g, q)), eps[g], ees[g], zparts[g])
            if q == NCHUNK - 1:
                emit_group_tail(g, ees[g], zparts[g])

        def unit_ctx(v):
            g, i = divmod(v, 4)
            if i == 0:
                ctxgs[g] = smallp.tile([128, 4], f32, tag="ctxg", name="ctxg")
            emit_ctx_batch(g, i, hps[g], ctxgs[g])
            if i == 3:
                nc.scalar.dma_start(
                    out=ctxu_d.ap()[:, 4 * g : 4 * g + 4], in_=ctxgs[g][:, :]
                )

        for u in range(NU + 8):
            if u < NU:
                unit_conv(u)
            if 1 <= u < NU + 1:
                unit_T(u - 1)
            if 2 <= u < NU + 2:
                unit_e(u - 2)
            # eb broadcast prefetch at 4g+6+i, ctx compute at 4g+8+i
            if u >= 6 and u - 6 < NG * 4:
                emit_eb_prefetch(u - 6, ees[(u - 6) // 4])
            if u >= 8 and u - 8 < NG * 4:
                unit_ctx(u - 8)

    nc.compile()
    _CACHE["prog"] = nc
    return nc


def _host_consts(emb, conv_w, conv_b, W_a, v_a):
    import ml_dtypes

    bf = ml_dtypes.bfloat16
    # permutation: d' = s*32 + c  <->  d = c*4 + s
    perm = np.array([(dp % 32) * 4 + dp // 32 for dp in range(128)], dtype=np.int64)
    G = np.einsum("cik,vi->kcv", conv_w.astype(np.float64), emb.astype(np.float64))
    G = G.astype(np.float32)  # [3, 32, 26]
    G[0] += conv_b.astype(np.float32)[:, None]
    gt = np.zeros((128, 3, 32), dtype=bf)
    for i in range(4):
        for k in range(3):
            gt[32 * i : 32 * i + 26, k, :] = G[k].T.astype(bf)  # [26, 32]
    W_p = W_a[np.ix_(perm, perm)].astype(np.float32)
    wat = np.ascontiguousarray(W_p.T).astype(bf)
    vcol = np.ascontiguousarray(
        np.repeat(v_a[perm].astype(np.float32)[:, None], 32, axis=1)
    ).astype(bf)
    return perm, gt, wat, vcol


def _in_maps(input_seq, emb, conv_w, conv_b, W_a, v_a):
    import ml_dtypes

    perm, gt, wat, vcol = _host_consts(emb, conv_w, conv_b, W_a, v_a)
    seq = np.asarray(input_seq).astype(np.int32)
    sym = np.arange(32, dtype=np.int32)[None, :, None]
    maps = []
    for c in range(NCORES):
        ohc = (seq[BPC * c : BPC * (c + 1), None, :] == sym).astype(
            ml_dtypes.float8_e4m3
        )
        maps.append(
            {
                "oh": ohc,
                "gt": gt,
                "wat": wat,
                "vcol": vcol,
            }
        )
    return perm, maps


def _assemble(perm, results):
    out = np.empty((B, D), dtype=np.float32)
    for c, res in enumerate(results):
        ctxu = res["ctxu"]  # [128, 16] d'-major, unnormalized
        zsum = res["zsum"][:, 0]  # [16]
        ctx_p = (ctxu / zsum[None, :]).T  # [16, 128] in d'-space
        blk = out[BPC * c : BPC * (c + 1)]
        blk[:, perm] = ctx_p
    return out


def kernel(input_seq, emb, conv_w, conv_b, W_a, v_a):
    from concourse import bass_utils

    nc = _build_program()
    perm, maps = _in_maps(input_seq, emb, conv_w, conv_b, W_a, v_a)
    res = bass_utils.run_bass_kernel_spmd(nc, maps, core_ids=list(range(NCORES)))
    return _assemble(perm, res.results)
